# revision 13
# baseline (speedup 1.0000x reference)
"""Segment mean-pool (LocalPooling1D) Trainium2 Bass kernel.

x [32, 8192, 256] f32, x_pos [32, 65] sorted int32 boundaries -> y [32, 64, 256].
y[b, j] = mean(x[b, x_pos[b,j]:x_pos[b,j+1]]), empty segments -> 0.

Strategy: data-parallel over batch, 4 rows per core on 8 cores. Token-to-segment
indicators are built on DVE from iota patterns; segment sums accumulate on the
TensorEngine as psum += ind.T @ x.

Perf notes:
- x is loaded with the token axis interleaved as t = b*(128*blk) + p*blk + k so
  each SBUF partition line is ONE contiguous HBM chunk (large DMA descriptors
  at line rate) instead of blk scattered 1 KiB chunks.
- x is cast f32 -> bf16 *during* the DMA (SWDGE datapath cast, no engine cost)
  and indicators are built in bf16, so matmuls run at 1 PE cycle/row instead of
  4 for fp32. This keeps the PE well ahead of the HBM stream (the fp32 version
  was PE-bound in steady state and kept re-triggering the HAM clock gate).
- indicator build for row r+1 is issued before the PSUM scale of row r so DVE
  never serializes the PE across row boundaries.
- pos loads / y stores ride the otherwise idle HWDGE (sync) queue; the first x
  DMAs are pre-issued ahead of the iota setup so HBM streaming starts at t=0.
"""

import os
import sys

import numpy as np

sys.path.insert(0, "/opt/trn_rl_repo")

import concourse.bacc as bacc
import concourse.bass as bass
import concourse.tile as tile
from concourse import mybir
from concourse.bass_utils import run_bass_kernel_spmd

dt = mybir.dt
Alu = mybir.AluOpType

# Problem constants (hardcoded per harness contract).
B, T, C, P = 32, 8192, 256, 65
NSEG = P - 1
NCORES = 8
R = B // NCORES          # batch rows per core
TOK = 128                # tokens per matmul tile (K)
KTILES = T // TOK        # 64 matmul tiles per row

CFG = {
    "blk": int(os.environ.get("KB_BLK", "16")),           # token-tiles per x DMA
    "col_pack": os.environ.get("KB_COLPACK", "1") == "1", # even/odd PE col groups
    "cast16": os.environ.get("KB_CAST16", "1") == "1",    # bf16 cast-DMA path
    "hybrid": os.environ.get("KB_HYBRID", "0") == "1",    # f32 blocks on HWDGE too
    "x_bufs": int(os.environ.get("KB_XBUFS", "12")),
    "ind_bufs": int(os.environ.get("KB_INDBUFS", "2")),
    "psum_bufs": int(os.environ.get("KB_PSUMBUFS", "2")),
    "pre_dma": int(os.environ.get("KB_PREDMA", "3")),     # x DMAs issued pre-iota
}


def build_program(cfg=CFG):
    blk = cfg["blk"]
    nblk = KTILES // blk
    col_pack = cfg["col_pack"]
    cast16 = cfg["cast16"]
    x_dt = dt.bfloat16 if cast16 else dt.float32

    hybrid = cfg["hybrid"] and cast16

    nc = bacc.Bacc("TRN2", target_bir_lowering=False, debug=False)

    x_d = nc.dram_tensor("x", [R, T, C], dt.float32, kind="ExternalInput")
    pos_d = nc.dram_tensor("x_pos", [R, P], dt.int32, kind="ExternalInput")
    y_d = nc.dram_tensor("y", [R, NSEG, C], dt.float32, kind="ExternalOutput")

    # Hybrid: odd blocks ride the two HWDGE rings as plain f32 (fp32 matmuls);
    # even blocks stay on the SWDGE cast path. Each SDMA engine then has 3
    # internal queues in flight, hiding HBM latency spikes under contention.
    def blk_is_f32(b):
        return hybrid and (b % 2 == 1)

    with tile.TileContext(nc) as tc:
        with (
            tc.tile_pool(name="const", bufs=1) as constp,
            tc.tile_pool(name="xp", bufs=6 if hybrid else cfg["x_bufs"]) as xp,
            tc.tile_pool(name="xfp", bufs=5) as xfp,
            tc.tile_pool(name="indp", bufs=cfg["ind_bufs"]) as indp,
            tc.tile_pool(name="smallp", bufs=1) as smallp,
            tc.tile_pool(name="outp", bufs=2) as outp,
            tc.tile_pool(name="psp", bufs=cfg["psum_bufs"], space="PSUM") as psp,
        ):
            # x row view with token axis t = b*(128*blk) + p*blk + k: the
            # partition line of block b is one contiguous blk*C*4-byte HBM
            # chunk.
            def x_dma(r, b):
                xr = x_d[r].rearrange("(b p k) c -> b p k c", p=TOK, k=blk)
                if blk_is_f32(b):
                    xt = xfp.tile([TOK, blk * C], dt.float32)
                    eng = nc.scalar if b % 4 == 3 else nc.sync
                elif cast16:
                    xt = xp.tile([TOK, blk * C], x_dt)
                    eng = nc.gpsimd          # SWDGE: casts f32->bf16 in-flight
                else:
                    xt = xp.tile([TOK, blk * C], x_dt)
                    eng = nc.scalar if b % 2 else nc.sync
                xt_v = xt[:].rearrange("p (k c) -> p k c", k=blk)
                eng.dma_start(xt_v, xr[b])
                return xt

            # Pre-issue the first x DMAs so HBM streaming starts immediately,
            # before the (gpsimd-engine) iota setup below.
            pre = {}
            for i in range(min(cfg["pre_dma"], nblk)):
                pre[(0, i)] = x_dma(0, i)

            # Token-tile base value per (b, k): 128*blk*b + k. Tiny [TOK,
            # KTILES] tile (values <= 8191, exact in f32) broadcast along the
            # segment axis inside the compare — avoids a huge 3-D iota on Q7.
            tio_b = constp.tile([TOK, nblk, blk], dt.float32)
            nc.gpsimd.iota(
                tio_b[:],
                pattern=[[TOK * blk, nblk], [1, blk]],
                base=0,
                channel_multiplier=0,
                allow_small_or_imprecise_dtypes=True,
            )
            tio_v = tio_b[:].rearrange("p b k -> p (b k)")
            # blk*p as a per-partition scalar (token index contribution of p).
            p_iota = constp.tile([TOK, 1], dt.float32)
            nc.gpsimd.iota(p_iota[:], pattern=[[1, 1]], base=0,
                           channel_multiplier=blk,
                           allow_small_or_imprecise_dtypes=True)

            # ---- pos prep for ALL rows up front (HWDGE load) ----
            # Broadcast the int32 row first, THEN cast on all 128 DVE lanes (a
            # single-partition cast would serialize on one lane, ~25x slower).
            pos_all = smallp.tile([1, R * P], dt.int32)
            nc.sync.dma_start(pos_all[:], pos_d.rearrange("r p -> (r p)")[None, :])
            pos_bi = smallp.tile([TOK, R * P], dt.int32)
            nc.gpsimd.partition_broadcast(pos_bi[:], pos_all[:])
            pos_bf = smallp.tile([TOK, R * P], dt.float32)
            nc.vector.tensor_copy(pos_bf[:], pos_bi[:])
            # pos_sh[p, (r,j)] = pos[r, j] - blk*p
            pos_sh = smallp.tile([TOK, R * P], dt.float32)
            nc.vector.tensor_scalar(pos_sh[:], pos_bf[:], p_iota[:], None,
                                    op0=Alu.subtract)

            def build_ind(r):
                """S[p,ti,j] = (pos[j] - blk*p <= tio[ti]); ind = S[j]-S[j+1].

                Comparisons run on f32 inputs (values <= 8192, exact); the 0/1
                outputs are stored in the matmul dtype (exact in bf16 too)."""
                S_all = indp.tile([TOK, KTILES, P], x_dt, tag="sall")
                nc.vector.tensor_tensor(
                    S_all[:],
                    pos_sh[:, r * P : (r + 1) * P][:, None, :]
                        .broadcast_to((TOK, KTILES, P)),
                    tio_v[:, :, None].broadcast_to((TOK, KTILES, P)),
                    op=Alu.is_le,
                )
                ind_all = indp.tile([TOK, KTILES, NSEG], x_dt, tag="ind")
                nc.vector.tensor_tensor(
                    ind_all[:], S_all[:, :, 0:NSEG], S_all[:, :, 1:P], op=Alu.subtract
                )
                if not hybrid:
                    return ind_all, None
                ind_f = indp.tile([TOK, KTILES, NSEG], dt.float32, tag="indf")
                nc.vector.tensor_tensor(
                    ind_f[:], S_all[:, :, 0:NSEG], S_all[:, :, 1:P], op=Alu.subtract
                )
                return ind_all, ind_f

            ind_cur, indf_cur = build_ind(0)

            # counts -> 1/max(cnt, 1), partition-major [NSEG, R]. Emitted
            # after build_ind(0) so the DVE reaches S0 as early as possible
            # (recip isn't needed until the first PSUM scale).
            pos_lo = smallp.tile([NSEG, R], dt.int32)
            pos_hi = smallp.tile([NSEG, R], dt.int32)
            nc.sync.dma_start(pos_lo[:], pos_d[:, 0:NSEG].rearrange("r p -> p r"))
            nc.sync.dma_start(pos_hi[:], pos_d[:, 1:P].rearrange("r p -> p r"))
            cnt_f = smallp.tile([NSEG, R], dt.float32)
            nc.vector.tensor_tensor(cnt_f[:], pos_hi[:], pos_lo[:], op=Alu.subtract)
            cntc = smallp.tile([NSEG, R], dt.float32)
            nc.vector.tensor_scalar(cntc[:], cnt_f[:], 1.0, None, op0=Alu.max)
            recip = smallp.tile([NSEG, R], dt.float32)
            nc.vector.reciprocal(recip[:], cntc[:])
            for r in range(R):
                ps = psp.tile([2 * NSEG if col_pack else NSEG, C], dt.float32)
                for b in range(nblk):
                    xt = pre.pop((r, b), None)
                    if xt is None:
                        xt = x_dma(r, b)
                    for k in range(blk):
                        ti = b * blk + k
                        rhs = xt[:, k * C : (k + 1) * C]
                        src = indf_cur if blk_is_f32(b) else ind_cur
                        lhsT = src[:, ti, :]
                        if col_pack:
                            half = ti % 2
                            nc.tensor.matmul(
                                ps[half * NSEG : (half + 1) * NSEG, :], lhsT, rhs,
                                start=(ti == half), stop=(ti == KTILES - 2 + half),
                                tile_position=(0, half * NSEG),
                                skip_group_check=True,
                            )
                        else:
                            nc.tensor.matmul(
                                ps[:], lhsT, rhs,
                                start=(ti == 0), stop=(ti == KTILES - 1),
                            )

                # Issue next row's indicator build BEFORE this row's PSUM scale
                # so the DVE isn't blocked on PE completion.
                if r + 1 < R:
                    ind_cur, indf_cur = build_ind(r + 1)

                rrec = recip[:, r : r + 1]
                out_t = outp.tile([NSEG, C], dt.float32)
                if col_pack:
                    # DVE reads one PSUM operand per op: scale each half alone.
                    # The even column group stops one matmul earlier, so scale
                    # it first.
                    half_t = outp.tile([NSEG, C], dt.float32, tag="half")
                    nc.vector.tensor_scalar(
                        half_t[:], ps[0:NSEG, :], rrec, None, op0=Alu.mult
                    )
                    nc.vector.scalar_tensor_tensor(
                        out_t[:], ps[NSEG : 2 * NSEG, :], rrec, half_t[:],
                        op0=Alu.mult, op1=Alu.add,
                    )
                else:
                    nc.vector.tensor_scalar(out_t[:], ps[:], rrec, None, op0=Alu.mult)
                nc.sync.dma_start(y_d[r], out_t[:])

    nc.compile()
    return nc


_PROGRAM = None


def _get_program():
    global _PROGRAM
    if _PROGRAM is None:
        _PROGRAM = build_program()
    return _PROGRAM


def kernel(x, x_pos):
    x = np.ascontiguousarray(x, dtype=np.float32)
    x_pos = np.ascontiguousarray(x_pos, dtype=np.int32)
    nc = _get_program()
    in_maps = [
        {"x": x[c * R : (c + 1) * R], "x_pos": x_pos[c * R : (c + 1) * R]}
        for c in range(NCORES)
    ]
    res = run_bass_kernel_spmd(nc, in_maps, list(range(NCORES)))
    y = np.concatenate([res.results[c]["y"] for c in range(NCORES)], axis=0)
    return y.astype(np.float32)


# revision 23
# speedup vs baseline: 1.0555x; 1.0555x over previous
"""Segment mean-pool (LocalPooling1D) Trainium2 Bass kernel.

x [32, 8192, 256] f32, x_pos [32, 65] sorted int32 boundaries -> y [32, 64, 256].
y[b, j] = mean(x[b, x_pos[b,j]:x_pos[b,j+1]]), empty segments -> 0.

Strategy: data-parallel over batch, 4 rows per core on 8 cores. Token-to-segment
indicators are built on DVE from iota patterns; segment sums accumulate on the
TensorEngine as psum += ind.T @ x.

Perf notes:
- x is loaded with the token axis interleaved as t = b*(128*blk) + p*blk + k so
  each SBUF partition line is ONE contiguous HBM chunk (large DMA descriptors
  at line rate) instead of blk scattered 1 KiB chunks.
- x is cast f32 -> bf16 *during* the DMA (SWDGE datapath cast, no engine cost)
  and indicators are built in bf16, so matmuls run at 1 PE cycle/row instead of
  4 for fp32. This keeps the PE well ahead of the HBM stream (the fp32 version
  was PE-bound in steady state and kept re-triggering the HAM clock gate).
- indicator build for row r+1 is issued before the PSUM scale of row r so DVE
  never serializes the PE across row boundaries.
- pos loads / y stores ride the otherwise idle HWDGE (sync) queue; the first x
  DMAs are pre-issued ahead of the iota setup so HBM streaming starts at t=0.
"""

import os
import sys

import numpy as np

sys.path.insert(0, "/opt/trn_rl_repo")

import concourse.bacc as bacc
import concourse.bass as bass
import concourse.tile as tile
from concourse import mybir
from concourse.bass_utils import run_bass_kernel_spmd

dt = mybir.dt
Alu = mybir.AluOpType

# Problem constants (hardcoded per harness contract).
B, T, C, P = 32, 8192, 256, 65
NSEG = P - 1
NCORES = 8
R = B // NCORES          # batch rows per core
TOK = 128                # tokens per matmul tile (K)
KTILES = T // TOK        # 64 matmul tiles per row

CFG = {
    "blk": int(os.environ.get("KB_BLK", "16")),           # token-tiles per x DMA
    "col_pack": os.environ.get("KB_COLPACK", "1") == "1", # even/odd PE col groups
    "cast16": os.environ.get("KB_CAST16", "1") == "1",    # bf16 cast-DMA path
    "hybrid": os.environ.get("KB_HYBRID", "0") == "1",    # f32 blocks on HWDGE too
    "f32_first": os.environ.get("KB_F32FIRST", "1") == "1",  # block (0,0) on sync
    "tail_split": os.environ.get("KB_TAILSPLIT", "1") == "1",  # k-split last block
    "x_bufs": int(os.environ.get("KB_XBUFS", "12")),
    "ind_bufs": int(os.environ.get("KB_INDBUFS", "2")),
    "psum_bufs": int(os.environ.get("KB_PSUMBUFS", "2")),
    "pre_dma": int(os.environ.get("KB_PREDMA", "3")),     # x DMAs issued pre-iota
}


def build_program(cfg=CFG):
    blk = cfg["blk"]
    nblk = KTILES // blk
    col_pack = cfg["col_pack"]
    cast16 = cfg["cast16"]
    x_dt = dt.bfloat16 if cast16 else dt.float32

    hybrid = cfg["hybrid"] and cast16
    f32_first = cfg["f32_first"] and cast16 and not hybrid
    tail_split = cfg["tail_split"] and cast16 and blk % 2 == 0 and not hybrid

    nc = bacc.Bacc("TRN2", target_bir_lowering=False, debug=False)

    x_d = nc.dram_tensor("x", [R, T, C], dt.float32, kind="ExternalInput")
    pos_d = nc.dram_tensor("x_pos", [R, P], dt.int32, kind="ExternalInput")
    y_d = nc.dram_tensor("y", [R, NSEG, C], dt.float32, kind="ExternalOutput")

    # Hybrid: odd blocks ride the two HWDGE rings as plain f32 (fp32 matmuls);
    # even blocks stay on the SWDGE cast path.
    # f32_first: only block (0,0) goes f32 on the (otherwise idle at t=0) sync
    # HWDGE ring, which issues ~1.5 us before the first SWDGE emission.
    def blk_is_f32(r, b):
        return (hybrid and b % 2 == 1) or (f32_first and r == 0 and b == 0)

    with tile.TileContext(nc) as tc:
        with (
            tc.tile_pool(name="const", bufs=1) as constp,
            tc.tile_pool(name="xp", bufs=6 if hybrid else cfg["x_bufs"]) as xp,
            tc.tile_pool(name="xfp", bufs=5 if hybrid else 1) as xfp,
            tc.tile_pool(name="xtailp", bufs=1) as xtailp,
            tc.tile_pool(name="indp", bufs=cfg["ind_bufs"]) as indp,
            tc.tile_pool(name="smallp", bufs=1) as smallp,
            tc.tile_pool(name="outp", bufs=2) as outp,
            tc.tile_pool(name="psp", bufs=cfg["psum_bufs"], space="PSUM") as psp,
        ):
            # x row view with token axis t = b*(128*blk) + p*blk + k: the
            # partition line of block b is one contiguous blk*C*4-byte HBM
            # chunk.
            def x_dma(r, b):
                xr = x_d[r].rearrange("(b p k) c -> b p k c", p=TOK, k=blk)
                if blk_is_f32(r, b):
                    xt = xfp.tile([TOK, blk * C], dt.float32)
                    eng = nc.scalar if (hybrid and b % 4 == 3) else nc.sync
                elif cast16:
                    xt = xp.tile([TOK, blk * C], x_dt)
                    eng = nc.gpsimd          # SWDGE: casts f32->bf16 in-flight
                else:
                    xt = xp.tile([TOK, blk * C], x_dt)
                    eng = nc.scalar if b % 2 else nc.sync
                xt_v = xt[:].rearrange("p (k c) -> p k c", k=blk)
                eng.dma_start(xt_v, xr[b])
                return xt

            # Pre-issue the first x DMAs so HBM streaming starts immediately,
            # before the (gpsimd-engine) iota setup below.
            pre = {}
            for i in range(min(cfg["pre_dma"], nblk)):
                pre[(0, i)] = x_dma(0, i)

            # Token-tile base value per (b, k): 128*blk*b + k. Tiny [TOK,
            # KTILES] tile (values <= 8191, exact in f32) broadcast along the
            # segment axis inside the compare — avoids a huge 3-D iota on Q7.
            tio_b = constp.tile([TOK, nblk, blk], dt.float32)
            nc.gpsimd.iota(
                tio_b[:],
                pattern=[[TOK * blk, nblk], [1, blk]],
                base=0,
                channel_multiplier=0,
                allow_small_or_imprecise_dtypes=True,
            )
            tio_v = tio_b[:].rearrange("p b k -> p (b k)")
            # blk*p as a per-partition scalar (token index contribution of p).
            p_iota = constp.tile([TOK, 1], dt.float32)
            nc.gpsimd.iota(p_iota[:], pattern=[[1, 1]], base=0,
                           channel_multiplier=blk,
                           allow_small_or_imprecise_dtypes=True)

            # ---- pos prep for ALL rows up front (HWDGE load) ----
            # Broadcast the int32 row first, THEN cast on all 128 DVE lanes (a
            # single-partition cast would serialize on one lane, ~25x slower).
            # pos rides the scalar HWDGE ring: the sync ring may be busy with
            # the f32 first x block and HWDGE rings are FIFO per engine.
            pos_all = smallp.tile([1, R * P], dt.int32)
            nc.scalar.dma_start(pos_all[:], pos_d.rearrange("r p -> (r p)")[None, :])
            pos_bi = smallp.tile([TOK, R * P], dt.int32)
            nc.gpsimd.partition_broadcast(pos_bi[:], pos_all[:])
            pos_bf = smallp.tile([TOK, R * P], dt.float32)
            nc.vector.tensor_copy(pos_bf[:], pos_bi[:])
            # pos_sh[p, (r,j)] = pos[r, j] - blk*p
            pos_sh = smallp.tile([TOK, R * P], dt.float32)
            nc.vector.tensor_scalar(pos_sh[:], pos_bf[:], p_iota[:], None,
                                    op0=Alu.subtract)

            def build_ind(r):
                """S[p,ti,j] = (pos[j] - blk*p <= tio[ti]); ind = S[j]-S[j+1].

                Comparisons run on f32 inputs (values <= 8192, exact); the 0/1
                outputs are stored in the matmul dtype (exact in bf16 too)."""
                S_all = indp.tile([TOK, KTILES, P], x_dt, tag="sall")
                nc.vector.tensor_tensor(
                    S_all[:],
                    pos_sh[:, r * P : (r + 1) * P][:, None, :]
                        .broadcast_to((TOK, KTILES, P)),
                    tio_v[:, :, None].broadcast_to((TOK, KTILES, P)),
                    op=Alu.is_le,
                )
                ind_f = None
                if f32_first and r == 0:
                    # f32 indicator for block (0,0)'s tiles, emitted FIRST so
                    # the earliest matmuls unblock as soon as possible.
                    ind_f = indp.tile([TOK, blk, NSEG], dt.float32, tag="indf")
                    nc.vector.tensor_tensor(
                        ind_f[:], S_all[:, 0:blk, 0:NSEG], S_all[:, 0:blk, 1:P],
                        op=Alu.subtract,
                    )
                ind_all = indp.tile([TOK, KTILES, NSEG], x_dt, tag="ind")
                nc.vector.tensor_tensor(
                    ind_all[:], S_all[:, :, 0:NSEG], S_all[:, :, 1:P], op=Alu.subtract
                )
                if hybrid:
                    ind_f = indp.tile([TOK, KTILES, NSEG], dt.float32, tag="indf")
                    nc.vector.tensor_tensor(
                        ind_f[:], S_all[:, :, 0:NSEG], S_all[:, :, 1:P],
                        op=Alu.subtract,
                    )
                return ind_all, ind_f

            ind_cur, indf_cur = build_ind(0)

            # counts -> 1/max(cnt, 1), partition-major [NSEG, R]. Emitted
            # after build_ind(0) so the DVE reaches S0 as early as possible
            # (recip isn't needed until the first PSUM scale).
            pos_lo = smallp.tile([NSEG, R], dt.int32)
            pos_hi = smallp.tile([NSEG, R], dt.int32)
            nc.sync.dma_start(pos_lo[:], pos_d[:, 0:NSEG].rearrange("r p -> p r"))
            nc.sync.dma_start(pos_hi[:], pos_d[:, 1:P].rearrange("r p -> p r"))
            cnt_f = smallp.tile([NSEG, R], dt.float32)
            nc.vector.tensor_tensor(cnt_f[:], pos_hi[:], pos_lo[:], op=Alu.subtract)
            cntc = smallp.tile([NSEG, R], dt.float32)
            nc.vector.tensor_scalar(cntc[:], cnt_f[:], 1.0, None, op0=Alu.max)
            recip = smallp.tile([NSEG, R], dt.float32)
            nc.vector.reciprocal(recip[:], cntc[:])
            for r in range(R):
                ps = psp.tile([2 * NSEG if col_pack else NSEG, C], dt.float32)
                for b in range(nblk):
                    last_blk = tail_split and r == R - 1 and b == nblk - 1
                    xt = pre.pop((r, b), None)
                    if xt is None and not last_blk:
                        xt = x_dma(r, b)
                    if last_blk:
                        # Split the final block's DMA by k-halves so the first
                        # half's matmuls overlap the second half's transfer
                        # (shrinks the post-last-byte tail).
                        h = blk // 2
                        xr = x_d[r].rearrange("(b p k) c -> b p k c", p=TOK, k=blk)
                        xta = xtailp.tile([TOK, h * C], x_dt, tag="xa")
                        xtb = xtailp.tile([TOK, h * C], x_dt, tag="xb")
                        nc.gpsimd.dma_start(
                            xta[:].rearrange("p (k c) -> p k c", k=h), xr[b][:, 0:h])
                        nc.gpsimd.dma_start(
                            xtb[:].rearrange("p (k c) -> p k c", k=h), xr[b][:, h:blk])
                    for k in range(blk):
                        ti = b * blk + k
                        if last_blk:
                            h = blk // 2
                            rhs = (xta[:, k * C : (k + 1) * C] if k < h
                                   else xtb[:, (k - h) * C : (k - h + 1) * C])
                        else:
                            rhs = xt[:, k * C : (k + 1) * C]
                        if blk_is_f32(r, b):
                            src_ti = ti if hybrid else k
                            src = indf_cur
                        else:
                            src_ti = ti
                            src = ind_cur
                        lhsT = src[:, src_ti, :]
                        if col_pack:
                            half = ti % 2
                            nc.tensor.matmul(
                                ps[half * NSEG : (half + 1) * NSEG, :], lhsT, rhs,
                                start=(ti == half), stop=(ti == KTILES - 2 + half),
                                tile_position=(0, half * NSEG),
                                skip_group_check=True,
                            )
                        else:
                            nc.tensor.matmul(
                                ps[:], lhsT, rhs,
                                start=(ti == 0), stop=(ti == KTILES - 1),
                            )

                # Issue next row's indicator build BEFORE this row's PSUM scale
                # so the DVE isn't blocked on PE completion.
                if r + 1 < R:
                    ind_cur, indf_cur = build_ind(r + 1)

                rrec = recip[:, r : r + 1]
                out_t = outp.tile([NSEG, C], dt.float32)
                if col_pack:
                    # DVE reads one PSUM operand per op: scale each half alone.
                    # The even column group stops one matmul earlier, so scale
                    # it first.
                    half_t = outp.tile([NSEG, C], dt.float32, tag="half")
                    nc.vector.tensor_scalar(
                        half_t[:], ps[0:NSEG, :], rrec, None, op0=Alu.mult
                    )
                    nc.vector.scalar_tensor_tensor(
                        out_t[:], ps[NSEG : 2 * NSEG, :], rrec, half_t[:],
                        op0=Alu.mult, op1=Alu.add,
                    )
                else:
                    nc.vector.tensor_scalar(out_t[:], ps[:], rrec, None, op0=Alu.mult)
                nc.sync.dma_start(y_d[r], out_t[:])

    nc.compile()
    return nc


_PROGRAM = None


def _get_program():
    global _PROGRAM
    if _PROGRAM is None:
        _PROGRAM = build_program()
    return _PROGRAM


def kernel(x, x_pos):
    x = np.ascontiguousarray(x, dtype=np.float32)
    x_pos = np.ascontiguousarray(x_pos, dtype=np.int32)
    nc = _get_program()
    in_maps = [
        {"x": x[c * R : (c + 1) * R], "x_pos": x_pos[c * R : (c + 1) * R]}
        for c in range(NCORES)
    ]
    res = run_bass_kernel_spmd(nc, in_maps, list(range(NCORES)))
    y = np.concatenate([res.results[c]["y"] for c in range(NCORES)], axis=0)
    return y.astype(np.float32)


# revision 29
# speedup vs baseline: 1.1151x; 1.0565x over previous
"""Segment mean-pool (LocalPooling1D) Trainium2 Bass kernel.

x [32, 8192, 256] f32, x_pos [32, 65] sorted int32 boundaries -> y [32, 64, 256].
y[b, j] = mean(x[b, x_pos[b,j]:x_pos[b,j+1]]), empty segments -> 0.

Strategy: data-parallel over batch, 4 rows per core on 8 cores. Token-to-segment
indicators are built on DVE from iota patterns; segment sums accumulate on the
TensorEngine as psum += ind.T @ x.

Perf notes:
- x is loaded with the token axis interleaved as t = b*(128*blk) + p*blk + k so
  each SBUF partition line is ONE contiguous HBM chunk (large DMA descriptors
  at line rate) instead of blk scattered 1 KiB chunks.
- x is cast f32 -> bf16 *during* the DMA (SWDGE datapath cast, no engine cost)
  and indicators are built in bf16, so matmuls run at 1 PE cycle/row instead of
  4 for fp32. This keeps the PE well ahead of the HBM stream (the fp32 version
  was PE-bound in steady state and kept re-triggering the HAM clock gate).
- indicator build for row r+1 is issued before the PSUM scale of row r so DVE
  never serializes the PE across row boundaries.
- pos loads / y stores ride the otherwise idle HWDGE (sync) queue; the first x
  DMAs are pre-issued ahead of the iota setup so HBM streaming starts at t=0.
"""

import os
import sys

import numpy as np

sys.path.insert(0, "/opt/trn_rl_repo")

import concourse.bacc as bacc
import concourse.bass as bass
import concourse.tile as tile
from concourse import mybir
from concourse.bass_utils import run_bass_kernel_spmd

dt = mybir.dt
Alu = mybir.AluOpType

# Problem constants (hardcoded per harness contract).
B, T, C, P = 32, 8192, 256, 65
NSEG = P - 1
NCORES = 8
R = B // NCORES          # batch rows per core
TOK = 128                # tokens per matmul tile (K)
KTILES = T // TOK        # 64 matmul tiles per row

CFG = {
    "blk": int(os.environ.get("KB_BLK", "16")),           # token-tiles per x DMA
    "col_pack": os.environ.get("KB_COLPACK", "1") == "1", # even/odd PE col groups
    "cast16": os.environ.get("KB_CAST16", "1") == "1",    # bf16 cast-DMA path
    "hybrid": os.environ.get("KB_HYBRID", "0") == "1",    # f32 blocks on HWDGE too
    "f32_first": os.environ.get("KB_F32FIRST", "1") == "1",  # block (0,0) on sync
    "tail_split": os.environ.get("KB_TAILSPLIT", "1") == "1",  # k-split last block
    "act_cast": os.environ.get("KB_ACTCAST", "0") == "1",  # odd blocks f32+ACT cast
    "x_bufs": int(os.environ.get("KB_XBUFS", "12")),
    "ind_bufs": int(os.environ.get("KB_INDBUFS", "2")),
    "psum_bufs": int(os.environ.get("KB_PSUMBUFS", "2")),
    "pre_dma": int(os.environ.get("KB_PREDMA", "3")),     # x DMAs issued pre-iota
}


def build_program(cfg=CFG):
    blk = cfg["blk"]
    nblk = KTILES // blk
    col_pack = cfg["col_pack"]
    cast16 = cfg["cast16"]
    x_dt = dt.bfloat16 if cast16 else dt.float32

    hybrid = cfg["hybrid"] and cast16
    act_cast = cfg["act_cast"] and cast16 and not hybrid
    f32_first = cfg["f32_first"] and cast16 and not hybrid
    tail_split = cfg["tail_split"] and cast16 and blk % 2 == 0 and not hybrid

    nc = bacc.Bacc("TRN2", target_bir_lowering=False, debug=False)

    x_d = nc.dram_tensor("x", [R, T, C], dt.float32, kind="ExternalInput")
    pos_d = nc.dram_tensor("x_pos", [R, P], dt.int32, kind="ExternalInput")
    y_d = nc.dram_tensor("y", [R, NSEG, C], dt.float32, kind="ExternalOutput")

    # Hybrid: odd blocks ride the two HWDGE rings as plain f32 (fp32 matmuls);
    # even blocks stay on the SWDGE cast path.
    # f32_first: only block (0,0) goes f32 on the (otherwise idle at t=0) sync
    # HWDGE ring, which issues ~1.5 us before the first SWDGE emission.
    def blk_is_f32(r, b):
        return (hybrid and b % 2 == 1) or (f32_first and r == 0 and b == 0)

    with tile.TileContext(nc) as tc:
        with (
            tc.tile_pool(name="const", bufs=1) as constp,
            tc.tile_pool(name="xp",
                         bufs=6 if hybrid else (10 if act_cast else cfg["x_bufs"])) as xp,
            tc.tile_pool(name="xfp", bufs=5 if hybrid else (3 if act_cast else 1)) as xfp,
            tc.tile_pool(name="xtailp", bufs=1) as xtailp,
            tc.tile_pool(name="indp", bufs=cfg["ind_bufs"]) as indp,
            tc.tile_pool(name="smallp", bufs=1) as smallp,
            tc.tile_pool(name="outp", bufs=2) as outp,
            tc.tile_pool(name="psp", bufs=cfg["psum_bufs"], space="PSUM") as psp,
        ):
            # x row view with token axis t = b*(128*blk) + p*blk + k: the
            # partition line of block b is one contiguous blk*C*4-byte HBM
            # chunk.
            def x_dma(r, b):
                xr = x_d[r].rearrange("(b p k) c -> b p k c", p=TOK, k=blk)
                if blk_is_f32(r, b):
                    pool = xfp if hybrid else xtailp
                    xt = pool.tile([TOK, blk * C], dt.float32, tag="f0")
                    eng = nc.scalar if (hybrid and b % 4 == 3) else nc.sync
                elif act_cast and b % 2 == 1:
                    # f32 via HWDGE (ring diversity vs the SWDGE-only path),
                    # cast to bf16 on the otherwise idle ACT engine.
                    xf = xfp.tile([TOK, blk * C], dt.float32, tag="stg")
                    eng = nc.sync if b % 4 == 1 else nc.scalar
                    eng.dma_start(xf[:].rearrange("p (k c) -> p k c", k=blk), xr[b])
                    xt = xp.tile([TOK, blk * C], x_dt)
                    nc.scalar.copy(xt[:], xf[:])
                    return xt
                elif cast16:
                    xt = xp.tile([TOK, blk * C], x_dt)
                    eng = nc.gpsimd          # SWDGE: casts f32->bf16 in-flight
                else:
                    xt = xp.tile([TOK, blk * C], x_dt)
                    eng = nc.scalar if b % 2 else nc.sync
                xt_v = xt[:].rearrange("p (k c) -> p k c", k=blk)
                eng.dma_start(xt_v, xr[b])
                return xt

            # Pre-issue the first x DMAs so HBM streaming starts immediately,
            # before the (gpsimd-engine) iota setup below.
            pre = {}
            for i in range(min(cfg["pre_dma"], nblk)):
                pre[(0, i)] = x_dma(0, i)

            # Token-tile base value per (b, k): 128*blk*b + k. Tiny [TOK,
            # KTILES] tile (values <= 8191, exact in f32) broadcast along the
            # segment axis inside the compare — avoids a huge 3-D iota on Q7.
            tio_b = constp.tile([TOK, nblk, blk], dt.float32)
            nc.gpsimd.iota(
                tio_b[:],
                pattern=[[TOK * blk, nblk], [1, blk]],
                base=0,
                channel_multiplier=0,
                allow_small_or_imprecise_dtypes=True,
            )
            tio_v = tio_b[:].rearrange("p b k -> p (b k)")
            # blk*p as a per-partition scalar (token index contribution of p).
            p_iota = constp.tile([TOK, 1], dt.float32)
            nc.gpsimd.iota(p_iota[:], pattern=[[1, 1]], base=0,
                           channel_multiplier=blk,
                           allow_small_or_imprecise_dtypes=True)

            # ---- pos prep for ALL rows up front (HWDGE load) ----
            # Broadcast the int32 row first, THEN cast on all 128 DVE lanes (a
            # single-partition cast would serialize on one lane, ~25x slower).
            # pos rides the scalar HWDGE ring: the sync ring may be busy with
            # the f32 first x block and HWDGE rings are FIFO per engine.
            pos_all = smallp.tile([1, R * P], dt.int32)
            nc.scalar.dma_start(pos_all[:], pos_d.rearrange("r p -> (r p)")[None, :])
            pos_bi = smallp.tile([TOK, R * P], dt.int32)
            nc.gpsimd.partition_broadcast(pos_bi[:], pos_all[:])
            pos_bf = smallp.tile([TOK, R * P], dt.float32)
            nc.vector.tensor_copy(pos_bf[:], pos_bi[:])
            # pos_sh[p, (r,j)] = pos[r, j] - blk*p
            pos_sh = smallp.tile([TOK, R * P], dt.float32)
            nc.vector.tensor_scalar(pos_sh[:], pos_bf[:], p_iota[:], None,
                                    op0=Alu.subtract)

            def build_ind(r):
                """S[p,ti,j] = (pos[j] - blk*p <= tio[ti]); ind = S[j]-S[j+1].

                Comparisons run on f32 inputs (values <= 8192, exact); the 0/1
                outputs are stored in the matmul dtype (exact in bf16 too)."""
                S_all = indp.tile([TOK, KTILES, P], x_dt, tag="sall")
                nc.vector.tensor_tensor(
                    S_all[:],
                    pos_sh[:, r * P : (r + 1) * P][:, None, :]
                        .broadcast_to((TOK, KTILES, P)),
                    tio_v[:, :, None].broadcast_to((TOK, KTILES, P)),
                    op=Alu.is_le,
                )
                ind_f = None
                if f32_first and r == 0:
                    # f32 indicator for block (0,0)'s tiles, emitted FIRST so
                    # the earliest matmuls unblock as soon as possible.
                    ind_f = indp.tile([TOK, blk, NSEG], dt.float32, tag="indf")
                    nc.vector.tensor_tensor(
                        ind_f[:], S_all[:, 0:blk, 0:NSEG], S_all[:, 0:blk, 1:P],
                        op=Alu.subtract,
                    )
                ind_all = indp.tile([TOK, KTILES, NSEG], x_dt, tag="ind")
                nc.vector.tensor_tensor(
                    ind_all[:], S_all[:, :, 0:NSEG], S_all[:, :, 1:P], op=Alu.subtract
                )
                if hybrid:
                    ind_f = indp.tile([TOK, KTILES, NSEG], dt.float32, tag="indf")
                    nc.vector.tensor_tensor(
                        ind_f[:], S_all[:, :, 0:NSEG], S_all[:, :, 1:P],
                        op=Alu.subtract,
                    )
                return ind_all, ind_f

            ind_cur, indf_cur = build_ind(0)

            # counts -> 1/max(cnt, 1), partition-major [NSEG, R]. Emitted
            # after build_ind(0) so the DVE reaches S0 as early as possible
            # (recip isn't needed until the first PSUM scale).
            pos_lo = smallp.tile([NSEG, R], dt.int32)
            pos_hi = smallp.tile([NSEG, R], dt.int32)
            nc.sync.dma_start(pos_lo[:], pos_d[:, 0:NSEG].rearrange("r p -> p r"))
            nc.sync.dma_start(pos_hi[:], pos_d[:, 1:P].rearrange("r p -> p r"))
            cnt_f = smallp.tile([NSEG, R], dt.float32)
            nc.vector.tensor_tensor(cnt_f[:], pos_hi[:], pos_lo[:], op=Alu.subtract)
            cntc = smallp.tile([NSEG, R], dt.float32)
            nc.vector.tensor_scalar(cntc[:], cnt_f[:], 1.0, None, op0=Alu.max)
            recip = smallp.tile([NSEG, R], dt.float32)
            nc.vector.reciprocal(recip[:], cntc[:])
            for r in range(R):
                ps = psp.tile([2 * NSEG if col_pack else NSEG, C], dt.float32)
                for b in range(nblk):
                    last_blk = tail_split and r == R - 1 and b == nblk - 1
                    xt = pre.pop((r, b), None)
                    if xt is None and not last_blk:
                        xt = x_dma(r, b)
                    if last_blk:
                        # Split the final block's DMA by k-halves so the first
                        # half's matmuls overlap the second half's transfer
                        # (shrinks the post-last-byte tail).
                        h = blk // 2
                        xr = x_d[r].rearrange("(b p k) c -> b p k c", p=TOK, k=blk)
                        xta = xtailp.tile([TOK, h * C], x_dt, tag="xa")
                        xtb = xtailp.tile([TOK, h * C], x_dt, tag="xb")
                        nc.gpsimd.dma_start(
                            xta[:].rearrange("p (k c) -> p k c", k=h), xr[b][:, 0:h])
                        nc.gpsimd.dma_start(
                            xtb[:].rearrange("p (k c) -> p k c", k=h), xr[b][:, h:blk])
                    for k in range(blk):
                        ti = b * blk + k
                        if last_blk:
                            h = blk // 2
                            rhs = (xta[:, k * C : (k + 1) * C] if k < h
                                   else xtb[:, (k - h) * C : (k - h + 1) * C])
                        else:
                            rhs = xt[:, k * C : (k + 1) * C]
                        if blk_is_f32(r, b):
                            src_ti = ti if hybrid else k
                            src = indf_cur
                        else:
                            src_ti = ti
                            src = ind_cur
                        lhsT = src[:, src_ti, :]
                        if col_pack:
                            half = ti % 2
                            nc.tensor.matmul(
                                ps[half * NSEG : (half + 1) * NSEG, :], lhsT, rhs,
                                start=(ti == half), stop=(ti == KTILES - 2 + half),
                                tile_position=(0, half * NSEG),
                                skip_group_check=True,
                            )
                        else:
                            nc.tensor.matmul(
                                ps[:], lhsT, rhs,
                                start=(ti == 0), stop=(ti == KTILES - 1),
                            )

                # Issue next row's indicator build BEFORE this row's PSUM scale
                # so the DVE isn't blocked on PE completion.
                if r + 1 < R:
                    ind_cur, indf_cur = build_ind(r + 1)

                rrec = recip[:, r : r + 1]
                out_t = outp.tile([NSEG, C], dt.float32)
                if col_pack:
                    # DVE reads one PSUM operand per op: scale each half alone.
                    # The even column group stops one matmul earlier, so scale
                    # it first — on ACT when that engine is idle, so the two
                    # halves scale concurrently.
                    half_t = outp.tile([NSEG, C], dt.float32, tag="half")
                    if act_cast:
                        nc.vector.tensor_scalar(
                            half_t[:], ps[0:NSEG, :], rrec, None, op0=Alu.mult
                        )
                    else:
                        nc.scalar.mul(half_t[:], ps[0:NSEG, :], rrec)
                    nc.vector.scalar_tensor_tensor(
                        out_t[:], ps[NSEG : 2 * NSEG, :], rrec, half_t[:],
                        op0=Alu.mult, op1=Alu.add,
                    )
                else:
                    nc.vector.tensor_scalar(out_t[:], ps[:], rrec, None, op0=Alu.mult)
                nc.sync.dma_start(y_d[r], out_t[:])

    nc.compile()
    return nc


_PROGRAM = None


def _get_program():
    global _PROGRAM
    if _PROGRAM is None:
        _PROGRAM = build_program()
    return _PROGRAM


def kernel(x, x_pos):
    x = np.ascontiguousarray(x, dtype=np.float32)
    x_pos = np.ascontiguousarray(x_pos, dtype=np.int32)
    nc = _get_program()
    in_maps = [
        {"x": x[c * R : (c + 1) * R], "x_pos": x_pos[c * R : (c + 1) * R]}
        for c in range(NCORES)
    ]
    res = run_bass_kernel_spmd(nc, in_maps, list(range(NCORES)))
    y = np.concatenate([res.results[c]["y"] for c in range(NCORES)], axis=0)
    return y.astype(np.float32)
